# revision 20
# baseline (speedup 1.0000x reference)
"""Trainium2 Bass kernel for nn_BasisAffinityGAT (B=8, N=512, D=R=128, K=8).

Math (matches reference.py):
    fused = concat(desc, nve) @ W_fuse + b_fuse                 [B,N,D]
    q = fused @ W_q[k];  kk = fused @ W_k[k]                    per basis
    e_q[b,k,n] = lrelu(q).a_q[k];  e_k likewise
    logits = e_q[:,:,:,None] + e_k[:,:,None,:], symmetrized
    alpha  = softmax(logits, -1); ema update; bias_log = log(clip(ema'))

Exact algebra used:
  * sym-logits[i,j] = 0.5*(s_i + s_j) with s = e_q + e_k, so the row
    softmax collapses: alpha[b,k,i,j] = softmax_j(0.5*s[b,k,:])[j],
    independent of i.
  * lrelu(x) = 0.6*x + 0.4*|x| (slope 0.2), so
    0.5*s[b,k,n] = fused[b,n,:] @ wlin[:,k]
                   + 0.2*(a_q[k] . |q_T|) + 0.2*(a_k[k] . |k_T|)
    with wlin[:,k] = 0.3*(W_q[k] @ a_q[k] + W_k[k] @ a_k[k]) host-folded.
  * bias_log content is batch-independent ([K,N,N] broadcast over B).

Sharding (8 cores, SPMD, zero cross-core communication): core m owns
basis k=m for ALL batches (an ncfw collective costs ~78us launch
latency on this runtime, so the K-sharded layout that keeps the batch
mean local wins).  Each batch is processed end-to-end (fused -> proj
-> e -> softmax -> PE broadcast -> alpha DMA) so the output DMA
starts ~10us in and streams continuously — the kernel is
output-bandwidth-bound as intended for this memory-regime problem.
The p-broadcast doubles as the softmax normalization (lhsT = 1/sum
replicated, rhs = exp(s)), and pbar accumulates on DVE straight from
the broadcast PSUM tiles (every partition row equals p_b).  All PE
matmuls run fp32r (fp22 multiplies, fp32 accumulate; walrus requires
fp32r-matmul operands to be produced as fp32r, hence the F32R tile
dtypes on DMA loads and ACT outputs).
"""

import sys

import numpy as np

if "/opt/trn_rl_repo" not in sys.path:
    sys.path.insert(0, "/opt/trn_rl_repo")

from contextlib import ExitStack

import concourse.bass as bass
import concourse.tile as tile
from concourse import bacc, mybir
from concourse.bass_utils import run_bass_kernel_spmd

B, N, D, K = 8, 512, 128, 8
R = D
MOM = 0.99
EPS = 1e-6
N_CORES = 8
F32 = mybir.dt.float32
F32R = mybir.dt.float32r
AF = mybir.ActivationFunctionType
ALU = mybir.AluOpType


def build():
    """Build the SPMD per-core Bass program (identical on all 8 cores)."""
    nc = bacc.Bacc("TRN2", target_bir_lowering=False, debug=False,
                   num_devices=N_CORES)

    # ---- per-core external tensors -------------------------------------
    # xTall[b,h,d,n]: h=0 desc[b].T, h=1 nve[b].T  (same array on all cores)
    xTall = nc.dram_tensor("xTall", [B, 2, D, N], F32R, kind="ExternalInput")
    wfuse = nc.dram_tensor("wfuse", [2, D, D], F32R, kind="ExternalInput")
    bfuse = nc.dram_tensor("bfuse", [D, 1], F32, kind="ExternalInput")
    wq = nc.dram_tensor("wq", [D, R], F32R, kind="ExternalInput")   # W_q[m]
    wk = nc.dram_tensor("wk", [D, R], F32R, kind="ExternalInput")   # W_k[m]
    aq1 = nc.dram_tensor("aq1", [R, 1], F32R, kind="ExternalInput")
    ak1 = nc.dram_tensor("ak1", [R, 1], F32R, kind="ExternalInput")
    wlin1 = nc.dram_tensor("wlin1", [D, 1], F32R, kind="ExternalInput")
    ema = nc.dram_tensor("ema", [N, N], F32, kind="ExternalInput")  # [m]
    alpha = nc.dram_tensor("alpha", [B, N, N], F32, kind="ExternalOutput")
    biaso = nc.dram_tensor("bias", [B, N, N], F32, kind="ExternalOutput")

    with ExitStack() as ctx:
        tc = ctx.enter_context(tile.TileContext(nc))
        const = ctx.enter_context(tc.tile_pool(name="const", bufs=1))
        work = ctx.enter_context(tc.tile_pool(name="work", bufs=2))
        absp = ctx.enter_context(tc.tile_pool(name="absp", bufs=4))
        psum = ctx.enter_context(tc.tile_pool(name="psum", bufs=1, space="PSUM"))

        # tiles declared up-front; loads emitted in latency-aware order
        wfuse_sb = const.tile([D, 2 * D], F32R)
        bfuse_sb = const.tile([D, 1], F32)
        wq_sb = const.tile([D, R], F32R)
        wk_sb = const.tile([D, R], F32R)
        aq_sb = const.tile([R, 1], F32R)
        ak_sb = const.tile([R, 1], F32R)
        wlin_sb = const.tile([D, 1], F32R)
        ones1_sb = const.tile([1, D], F32)
        ema_sb = const.tile([128, 4 * N], F32)

        # fusion weights + first batch first — they gate the first matmul
        nc.sync.dma_start(wfuse_sb[:].rearrange("d (h c) -> d h c", h=2),
                          wfuse.ap().rearrange("h d c -> d h c"))
        nc.sync.dma_start(bfuse_sb[:], bfuse[:])
        nc.vector.memset(ones1_sb[:], 1.0)
        rep_list = []

        for b in range(B):
            xb = work.tile([D, 2 * N], F32R, tag="xb", bufs=4)
            nc.sync.dma_start(
                xb[:].rearrange("d (h n) -> d h n", h=2),
                xTall[b].rearrange("h d n -> d h n"))
            if b == 0:
                nc.gpsimd.dma_start(wq_sb[:], wq[:])
                nc.gpsimd.dma_start(wk_sb[:], wk[:])
                nc.gpsimd.dma_start(aq_sb[:], aq1[:])
                nc.gpsimd.dma_start(ak_sb[:], ak1[:])
                nc.gpsimd.dma_start(wlin_sb[:], wlin1[:])
            psum_f = psum.tile([D, N], F32, tag="mm", bufs=3)
            nc.tensor.matmul(psum_f[:], wfuse_sb[:, 0:D], xb[:, 0:N],
                             start=True, stop=False)
            nc.tensor.matmul(psum_f[:], wfuse_sb[:, D:2 * D],
                             xb[:, N:2 * N], start=False, stop=True)
            fused_sb = absp.tile([D, N], F32R, tag="fused", bufs=3)
            nc.vector.tensor_scalar_add(fused_sb[:], psum_f[:], bfuse_sb[:])
            psum_s = psum.tile([1, N], F32, tag="ps", bufs=2)
            nc.tensor.matmul(psum_s[:], wlin_sb[:], fused_sb[:],
                             start=True, stop=False)
            psum_q = psum.tile([D, N], F32, tag="mm", bufs=3)
            nc.tensor.matmul(psum_q[:], wq_sb[:], fused_sb[:],
                             start=True, stop=True)
            absq = absp.tile([D, N], F32R, tag="abs", bufs=4)
            nc.scalar.activation(absq[:], psum_q[:], AF.Abs)
            nc.tensor.matmul(psum_s[:], aq_sb[:], absq[:],
                             start=False, stop=False)
            psum_k = psum.tile([D, N], F32, tag="mm", bufs=3)
            nc.tensor.matmul(psum_k[:], wk_sb[:], fused_sb[:],
                             start=True, stop=True)
            absk = absp.tile([D, N], F32R, tag="abs", bufs=4)
            nc.scalar.activation(absk[:], psum_k[:], AF.Abs)
            nc.tensor.matmul(psum_s[:], ak_sb[:], absk[:],
                             start=False, stop=True)

            # ---- softmax over free dim (no max-shift: |s| is O(1), exp
            # is safe in fp32 and softmax is shift-invariant) -------------
            expv = work.tile([1, N], F32R, tag="ex", bufs=6)
            sume = work.tile([1, 1], F32, tag="se", bufs=6)
            nc.scalar.activation(expv[:], psum_s[:], AF.Exp,
                                 scale=1.0, accum_out=sume[:])
            rsum = work.tile([1, 1], F32, tag="rs", bufs=6)
            nc.vector.reciprocal(rsum[:], sume[:])

            # ---- alpha[b, i, :] = p_b for all i ------------------------
            # broadcast via PE: lhsT = rsum replicated (ACT, fp32r) so the
            # matmul computes rsum*expv = p on all 128 partitions.
            rsum_rep = work.tile([1, D], F32R, tag="rr", bufs=6)
            nc.vector.tensor_scalar_mul(rsum_rep[:], ones1_sb[:], rsum[:])
            psum_rep = psum.tile([128, N], F32, tag="rep", bufs=3)
            nc.tensor.matmul(psum_rep[:], rsum_rep[:], expv[:],
                             start=True, stop=True)
            rep_t = work.tile([128, N], F32, tag="repsb", bufs=8)
            nc.vector.tensor_copy(rep_t[:], psum_rep[:])
            rep_list.append(rep_t)
            src = rep_t[:].rearrange(
                "p (o n) -> p o n", o=1).broadcast_to([128, 4, N])
            dst = alpha[b].rearrange("(p i) j -> p i j", p=128)
            nc.sync.dma_start(dst, src)
            if b == 0:
                nc.sync.dma_start(
                    ema_sb[:].rearrange("p (c n) -> p c n", c=4),
                    ema.ap().rearrange("(c p) n -> p c n", p=128))

        # ---- bias_log: pbar is LOCAL (partition-sum over batches) ------
        s01 = work.tile([128, N], F32, tag="t0", bufs=1)
        nc.vector.tensor_add(s01[:], rep_list[0][:], rep_list[1][:])
        s23 = work.tile([128, N], F32, tag="t1", bufs=1)
        nc.vector.tensor_add(s23[:], rep_list[2][:], rep_list[3][:])
        s45 = work.tile([128, N], F32, tag="t2", bufs=1)
        nc.vector.tensor_add(s45[:], rep_list[4][:], rep_list[5][:])
        s67 = work.tile([128, N], F32, tag="t3", bufs=1)
        nc.vector.tensor_add(s67[:], rep_list[6][:], rep_list[7][:])
        s03 = work.tile([128, N], F32, tag="t4", bufs=1)
        nc.vector.tensor_add(s03[:], s01[:], s23[:])
        s47 = work.tile([128, N], F32, tag="t5", bufs=1)
        nc.vector.tensor_add(s47[:], s45[:], s67[:])
        pbs_acc = work.tile([128, N], F32, tag="t6", bufs=1)
        nc.vector.scalar_tensor_tensor(
            pbs_acc[:], s03[:], 1.0, s47[:],
            op0=mybir.AluOpType.mult, op1=mybir.AluOpType.add)
        for c in range(4):
            u = work.tile([128, N], F32, tag="u", bufs=2)
            nc.vector.scalar_tensor_tensor(
                u[:], pbs_acc[:], 0.01 / B / MOM, ema_sb[:, bass.ts(c, N)],
                op0=mybir.AluOpType.mult, op1=mybir.AluOpType.add)
            v = work.tile([128, N], F32, tag="v", bufs=2)
            nc.vector.tensor_scalar_max(v[:], u[:], EPS / MOM)
            bias_t = work.tile([128, N], F32, tag="biassb", bufs=2)
            nc.scalar.activation(bias_t[:], v[:], AF.Ln, scale=MOM)
            src = bias_t[:].rearrange(
                "p (o n) -> p o n", o=1).broadcast_to([128, B, N])
            dst = biaso.ap().rearrange("b (c p) j -> c p b j", c=4)[c]
            nc.sync.dma_start(dst, src)

    nc.compile()
    return nc


_NC_CACHE = None


def _get_nc():
    global _NC_CACHE
    if _NC_CACHE is None:
        _NC_CACHE = build()
    return _NC_CACHE


def make_in_maps(desc_embeddings, name_value_embeddings, W_fuse, b_fuse,
                 W_q, W_k, a, alpha_ema):
    """Host-side sharding / weight prep -> per-core input dicts."""
    desc = np.asarray(desc_embeddings, np.float32)
    nve = np.asarray(name_value_embeddings, np.float32)
    W_fuse = np.asarray(W_fuse, np.float32)
    b_fuse = np.asarray(b_fuse, np.float32)
    W_q = np.asarray(W_q, np.float32)
    W_k = np.asarray(W_k, np.float32)
    a = np.asarray(a, np.float32)
    alpha_ema = np.asarray(alpha_ema, np.float32)

    a_q = a[:, :R, 0]                      # [K,R]
    a_k = a[:, R:, 0]                      # [K,R]
    wlin = 0.3 * (np.einsum("kdr,kr->kd", W_q, a_q)
                  + np.einsum("kdr,kr->kd", W_k, a_k))  # [K,D]

    # xTall[b] = [desc[b].T, nve[b].T] — shared across cores
    xTall = np.ascontiguousarray(
        np.stack([np.stack([desc[b].T, nve[b].T], axis=0)
                  for b in range(B)], axis=0))
    wfuse_stack = np.ascontiguousarray(W_fuse.reshape(2, D, D))
    bfuse_col = np.ascontiguousarray(b_fuse.reshape(D, 1))

    shared = dict(xTall=xTall, wfuse=wfuse_stack, bfuse=bfuse_col)
    in_maps = []
    for m in range(N_CORES):
        in_maps.append(dict(
            shared,
            wq=np.ascontiguousarray(W_q[m]),
            wk=np.ascontiguousarray(W_k[m]),
            aq1=np.ascontiguousarray(0.2 * a_q[m].reshape(R, 1)),
            ak1=np.ascontiguousarray(0.2 * a_k[m].reshape(R, 1)),
            wlin1=np.ascontiguousarray(wlin[m].reshape(D, 1)),
            ema=np.ascontiguousarray(alpha_ema[m])))
    return in_maps


def gather(results):
    alpha_full = np.stack([r["alpha"] for r in results], axis=1)
    bias_full = np.stack([r["bias"] for r in results], axis=1)
    return bias_full, alpha_full


def kernel(**inputs):
    nc = _get_nc()
    in_maps = make_in_maps(**inputs)
    res = run_bass_kernel_spmd(nc, in_maps, list(range(N_CORES)))
    return gather(res.results)


# revision 21
# speedup vs baseline: 1.0950x; 1.0950x over previous
"""Trainium2 Bass kernel for nn_BasisAffinityGAT (B=8, N=512, D=R=128, K=8).

Math (matches reference.py):
    fused = concat(desc, nve) @ W_fuse + b_fuse                 [B,N,D]
    q = fused @ W_q[k];  kk = fused @ W_k[k]                    per basis
    e_q[b,k,n] = lrelu(q).a_q[k];  e_k likewise
    logits = e_q[:,:,:,None] + e_k[:,:,None,:], symmetrized
    alpha  = softmax(logits, -1); ema update; bias_log = log(clip(ema'))

Exact algebra used:
  * sym-logits[i,j] = 0.5*(s_i + s_j) with s = e_q + e_k, so the row
    softmax collapses: alpha[b,k,i,j] = softmax_j(0.5*s[b,k,:])[j],
    independent of i.
  * lrelu(x) = 0.6*x + 0.4*|x| (slope 0.2), so
    0.5*s[b,k,n] = fused[b,n,:] @ wlin[:,k]
                   + 0.2*(a_q[k] . |q_T|) + 0.2*(a_k[k] . |k_T|)
    with wlin[:,k] = 0.3*(W_q[k] @ a_q[k] + W_k[k] @ a_k[k]) host-folded.
  * bias_log content is batch-independent ([K,N,N] broadcast over B).

Sharding (8 cores, SPMD, zero cross-core communication): core m owns
basis k=m for ALL batches (an ncfw collective costs ~78us launch
latency on this runtime, so the K-sharded layout that keeps the batch
mean local wins).  Each batch is processed end-to-end (fused -> proj
-> e -> softmax -> PE broadcast -> alpha DMA) so the output DMA
starts ~10us in and streams continuously — the kernel is
output-bandwidth-bound as intended for this memory-regime problem.
The p-broadcast doubles as the softmax normalization (lhsT = 1/sum
replicated, rhs = exp(s)), and pbar accumulates on DVE straight from
the broadcast PSUM tiles (every partition row equals p_b).  All PE
matmuls run fp32r (fp22 multiplies, fp32 accumulate; walrus requires
fp32r-matmul operands to be produced as fp32r, hence the F32R tile
dtypes on DMA loads and ACT outputs).
"""

import sys

import numpy as np

if "/opt/trn_rl_repo" not in sys.path:
    sys.path.insert(0, "/opt/trn_rl_repo")

from contextlib import ExitStack

import concourse.bass as bass
import concourse.tile as tile
from concourse import bacc, mybir
from concourse.bass_utils import run_bass_kernel_spmd

B, N, D, K = 8, 512, 128, 8
R = D
MOM = 0.99
EPS = 1e-6
N_CORES = 8
F32 = mybir.dt.float32
F32R = mybir.dt.float32r
AF = mybir.ActivationFunctionType
ALU = mybir.AluOpType


def build():
    """Build the SPMD per-core Bass program (identical on all 8 cores)."""
    nc = bacc.Bacc("TRN2", target_bir_lowering=False, debug=False,
                   num_devices=N_CORES)

    # ---- per-core external tensors -------------------------------------
    # xTall[b,h,d,n]: h=0 desc[b].T, h=1 nve[b].T  (same array on all cores)
    xTall = nc.dram_tensor("xTall", [B, 2, D, N], F32R, kind="ExternalInput")
    wfuse = nc.dram_tensor("wfuse", [2, D, D], F32R, kind="ExternalInput")
    bfuse = nc.dram_tensor("bfuse", [D, 1], F32, kind="ExternalInput")
    wq = nc.dram_tensor("wq", [D, R], F32R, kind="ExternalInput")   # W_q[m]
    wk = nc.dram_tensor("wk", [D, R], F32R, kind="ExternalInput")   # W_k[m]
    aq1 = nc.dram_tensor("aq1", [R, 1], F32R, kind="ExternalInput")
    ak1 = nc.dram_tensor("ak1", [R, 1], F32R, kind="ExternalInput")
    wlin1 = nc.dram_tensor("wlin1", [D, 1], F32R, kind="ExternalInput")
    ema = nc.dram_tensor("ema", [N, N], F32, kind="ExternalInput")  # [m]
    alpha = nc.dram_tensor("alpha", [B, N, N], F32, kind="ExternalOutput")
    biaso = nc.dram_tensor("bias", [B, N, N], F32, kind="ExternalOutput")

    with ExitStack() as ctx:
        tc = ctx.enter_context(tile.TileContext(nc))
        const = ctx.enter_context(tc.tile_pool(name="const", bufs=1))
        work = ctx.enter_context(tc.tile_pool(name="work", bufs=2))
        absp = ctx.enter_context(tc.tile_pool(name="absp", bufs=4))
        psum = ctx.enter_context(tc.tile_pool(name="psum", bufs=1, space="PSUM"))

        # tiles declared up-front; loads emitted in latency-aware order
        wfuse_sb = const.tile([D, 2 * D], F32R)
        bfuse_sb = const.tile([D, 1], F32)
        wq_sb = const.tile([D, R], F32R)
        wk_sb = const.tile([D, R], F32R)
        aq_sb = const.tile([R, 1], F32R)
        ak_sb = const.tile([R, 1], F32R)
        wlin_sb = const.tile([D, 1], F32R)
        ones1_sb = const.tile([1, D], F32)
        ema_sb = const.tile([128, 4 * N], F32)

        # fusion weights + first batch first — they gate the first matmul
        nc.sync.dma_start(wfuse_sb[:].rearrange("d (h c) -> d h c", h=2),
                          wfuse.ap().rearrange("h d c -> d h c"))
        nc.sync.dma_start(bfuse_sb[:], bfuse[:])
        nc.vector.memset(ones1_sb[:], 1.0)
        pbs_acc = const.tile([128, N], F32)

        for b in range(B):
            xb = work.tile([D, 2 * N], F32R, tag="xb", bufs=4)
            nc.sync.dma_start(
                xb[:].rearrange("d (h n) -> d h n", h=2),
                xTall[b].rearrange("h d n -> d h n"))
            if b == 0:
                nc.gpsimd.dma_start(wq_sb[:], wq[:])
                nc.gpsimd.dma_start(wk_sb[:], wk[:])
                nc.gpsimd.dma_start(aq_sb[:], aq1[:])
                nc.gpsimd.dma_start(ak_sb[:], ak1[:])
                nc.gpsimd.dma_start(wlin_sb[:], wlin1[:])
            psum_f = psum.tile([D, N], F32, tag="mm", bufs=3)
            nc.tensor.matmul(psum_f[:], wfuse_sb[:, 0:D], xb[:, 0:N],
                             start=True, stop=False)
            nc.tensor.matmul(psum_f[:], wfuse_sb[:, D:2 * D],
                             xb[:, N:2 * N], start=False, stop=True)
            fused_sb = absp.tile([D, N], F32R, tag="fused", bufs=3)
            nc.vector.tensor_scalar_add(fused_sb[:], psum_f[:], bfuse_sb[:])
            psum_s = psum.tile([1, N], F32, tag="ps", bufs=3)
            nc.tensor.matmul(psum_s[:], wlin_sb[:], fused_sb[:],
                             start=True, stop=False)
            psum_q = psum.tile([D, N], F32, tag="mm", bufs=3)
            nc.tensor.matmul(psum_q[:], wq_sb[:], fused_sb[:],
                             start=True, stop=True)
            absq = absp.tile([D, N], F32R, tag="abs", bufs=4)
            nc.scalar.activation(absq[:], psum_q[:], AF.Abs)
            nc.tensor.matmul(psum_s[:], aq_sb[:], absq[:],
                             start=False, stop=False)
            psum_k = psum.tile([D, N], F32, tag="mm", bufs=3)
            nc.tensor.matmul(psum_k[:], wk_sb[:], fused_sb[:],
                             start=True, stop=True)
            absk = absp.tile([D, N], F32R, tag="abs", bufs=4)
            nc.scalar.activation(absk[:], psum_k[:], AF.Abs)
            nc.tensor.matmul(psum_s[:], ak_sb[:], absk[:],
                             start=False, stop=True)

            # ---- softmax over free dim (no max-shift: |s| is O(1), exp
            # is safe in fp32 and softmax is shift-invariant) -------------
            expv = work.tile([1, N], F32R, tag="ex", bufs=6)
            sume = work.tile([1, 1], F32, tag="se", bufs=6)
            nc.scalar.activation(expv[:], psum_s[:], AF.Exp,
                                 scale=1.0, accum_out=sume[:])
            rsum = work.tile([1, 1], F32, tag="rs", bufs=6)
            nc.vector.reciprocal(rsum[:], sume[:])

            # ---- alpha[b, i, :] = p_b for all i ------------------------
            # broadcast via PE: lhsT = rsum replicated (ACT, fp32r) so the
            # matmul computes rsum*expv = p on all 128 partitions.
            rsum_rep = work.tile([1, D], F32R, tag="rr", bufs=6)
            nc.vector.tensor_scalar_mul(rsum_rep[:], ones1_sb[:], rsum[:])
            psum_rep = psum.tile([128, N], F32, tag="rep", bufs=2)
            nc.tensor.matmul(psum_rep[:], rsum_rep[:], expv[:],
                             start=True, stop=True)
            rep_t = work.tile([128, N], F32, tag="repsb", bufs=4)
            nc.vector.tensor_copy(rep_t[:], psum_rep[:])
            if b == 0:
                nc.vector.tensor_scalar_mul(pbs_acc[:], psum_rep[:],
                                            0.01 / B / MOM)
            else:
                nc.vector.scalar_tensor_tensor(
                    pbs_acc[:], psum_rep[:], 0.01 / B / MOM, pbs_acc[:],
                    op0=mybir.AluOpType.mult, op1=mybir.AluOpType.add)
            src = rep_t[:].rearrange(
                "p (o n) -> p o n", o=1).broadcast_to([128, 4, N])
            dst = alpha[b].rearrange("(p i) j -> p i j", p=128)
            nc.sync.dma_start(dst, src)
            if b == 0:
                nc.sync.dma_start(
                    ema_sb[:].rearrange("p (c n) -> p c n", c=4),
                    ema.ap().rearrange("(c p) n -> p c n", p=128))

        # ---- bias_log: pbar is LOCAL (partition-sum over batches) ------
        for c in range(4):
            u = work.tile([128, N], F32, tag="u", bufs=2)
            nc.vector.tensor_add(u[:], ema_sb[:, bass.ts(c, N)], pbs_acc[:])
            v = work.tile([128, N], F32, tag="v", bufs=2)
            nc.vector.tensor_scalar_max(v[:], u[:], EPS / MOM)
            bias_t = work.tile([128, N], F32, tag="biassb", bufs=2)
            nc.scalar.activation(bias_t[:], v[:], AF.Ln, scale=MOM)
            src = bias_t[:].rearrange(
                "p (o n) -> p o n", o=1).broadcast_to([128, B, N])
            dst = biaso.ap().rearrange("b (c p) j -> c p b j", c=4)[c]
            nc.sync.dma_start(dst, src)

    nc.compile()
    return nc


_NC_CACHE = None


def _get_nc():
    global _NC_CACHE
    if _NC_CACHE is None:
        _NC_CACHE = build()
    return _NC_CACHE


def make_in_maps(desc_embeddings, name_value_embeddings, W_fuse, b_fuse,
                 W_q, W_k, a, alpha_ema):
    """Host-side sharding / weight prep -> per-core input dicts."""
    desc = np.asarray(desc_embeddings, np.float32)
    nve = np.asarray(name_value_embeddings, np.float32)
    W_fuse = np.asarray(W_fuse, np.float32)
    b_fuse = np.asarray(b_fuse, np.float32)
    W_q = np.asarray(W_q, np.float32)
    W_k = np.asarray(W_k, np.float32)
    a = np.asarray(a, np.float32)
    alpha_ema = np.asarray(alpha_ema, np.float32)

    a_q = a[:, :R, 0]                      # [K,R]
    a_k = a[:, R:, 0]                      # [K,R]
    wlin = 0.3 * (np.einsum("kdr,kr->kd", W_q, a_q)
                  + np.einsum("kdr,kr->kd", W_k, a_k))  # [K,D]

    # xTall[b] = [desc[b].T, nve[b].T] — shared across cores
    xTall = np.ascontiguousarray(
        np.stack([np.stack([desc[b].T, nve[b].T], axis=0)
                  for b in range(B)], axis=0))
    wfuse_stack = np.ascontiguousarray(W_fuse.reshape(2, D, D))
    bfuse_col = np.ascontiguousarray(b_fuse.reshape(D, 1))

    shared = dict(xTall=xTall, wfuse=wfuse_stack, bfuse=bfuse_col)
    in_maps = []
    for m in range(N_CORES):
        in_maps.append(dict(
            shared,
            wq=np.ascontiguousarray(W_q[m]),
            wk=np.ascontiguousarray(W_k[m]),
            aq1=np.ascontiguousarray(0.2 * a_q[m].reshape(R, 1)),
            ak1=np.ascontiguousarray(0.2 * a_k[m].reshape(R, 1)),
            wlin1=np.ascontiguousarray(wlin[m].reshape(D, 1)),
            ema=np.ascontiguousarray(alpha_ema[m])))
    return in_maps


def gather(results):
    alpha_full = np.stack([r["alpha"] for r in results], axis=1)
    bias_full = np.stack([r["bias"] for r in results], axis=1)
    return bias_full, alpha_full


def kernel(**inputs):
    nc = _get_nc()
    in_maps = make_in_maps(**inputs)
    res = run_bass_kernel_spmd(nc, in_maps, list(range(N_CORES)))
    return gather(res.results)
